# revision 20
# baseline (speedup 1.0000x reference)
"""Causal self-attention (B=4, T=2048, C=1024, H=16) on 8 trn2 NeuronCores.

Sharding: (batch, head-group) -> core.  Core c handles batch b = c//2 and
head-group hg = c%2 (8 heads = 512 channels).  c_attn is column-parallel
(each core computes q/k/v only for its heads), c_proj is row-parallel
(each core computes a partial [T, C] output over its 512 channels); the
host sums the two partials per batch (the unshard step).

Per-core dataflow (all matmul inputs fp32r = full-rate 12-bit-mantissa):
  phase 1a  qkT[ch, T]  = [Wq; Wk] @ x^T          (stationary W, moving x^T)
  phase 1b  V[t, ch]    = x @ Wv^T                (stationary x^T, moving Wv^T)
            V' = [V | 1] per head (ones column -> softmax denominator)
  phase 2   per head-pair, per 512-wide q-chunk:
              S^T[k, q] = K Q^T   (K=64 contraction; even/odd heads at
                                   partition offsets 0/64 -> packed PE tiles)
              E = exp(S^T/8) with causal mask (upper-tri 0/1 multiply)
              Y'^T[65, q] += V'^T E  (row 64 = denominator)
              yT = Y'^T[0:64] * (1/denom)  (reciprocal + partition broadcast)
  phase 3   out[q, C] partial = yT^T @ Wp_slice^T, staged to DRAM.

No collectives; the host-side pairwise add is the row-parallel reduce.
"""

import numpy as np

import concourse.bass as bass
import concourse.mybir as mybir
import concourse.tile as tile
from concourse import bacc
from concourse.bass_utils import run_bass_kernel_spmd

F32 = mybir.dt.float32
F32R = mybir.dt.float32r
EXP = mybir.ActivationFunctionType.Exp

C = 1024  # embed dim
HL = 8  # local heads per core
HD = 64  # head dim
N_CORES = 8

TRACE = False  # set by test harness; enables NTFF profiling
last_results = None  # BassKernelResults of the most recent run

_module_cache = {}


def build_module(T=2048):
    """Emit the per-core Bass program (SPMD; all cores run this)."""
    KT = C // 128  # contraction k-tiles for the projections (8)
    NM = 2 * HL * HD // 128  # qk channel m-tiles (8): 0-3 q pairs, 4-7 k pairs
    TT = T // 128  # T k-tiles (16)
    NCH = T // 512  # 512-wide q chunks (4)
    NP = HL // 2  # head pairs (4)
    CL = HL * HD  # local channels (512)

    nc = bacc.Bacc("TRN2", target_bir_lowering=False, debug=False)

    xT_d = nc.dram_tensor("xT", [C, T], F32R, kind="ExternalInput")
    wqk_d = nc.dram_tensor("wqkT", [NM * C, 128], F32R, kind="ExternalInput")
    wv_d = nc.dram_tensor("wvT", [C, CL], F32R, kind="ExternalInput")
    wp_d = nc.dram_tensor("wpT", [CL, C], F32R, kind="ExternalInput")
    out_d = nc.dram_tensor("out_part", [T, C], F32, kind="ExternalOutput")

    with tile.TileContext(nc) as tc:
        # ---- persistent buffers --------------------------------------
        with tc.tile_pool(name="persist", bufs=1) as cp:
            qkT = cp.tile([128, NM, T], F32R, tag="qkT")  # [ch-tile, m, T]
            vbuf = cp.tile([128, TT, HL * (HD + 1)], F32R, tag="vbuf")  # V'
            tri = cp.tile([128, 128], F32, tag="tri")
            nc.gpsimd.memset(tri, 1.0)
            # keep where (col - p) >= 0: upper triangular incl. diagonal
            nc.gpsimd.affine_select(
                out=tri,
                in_=tri,
                compare_op=mybir.AluOpType.is_ge,
                fill=0.0,
                base=0,
                pattern=[[1, 128]],
                channel_multiplier=-1,
            )
            ones8 = cp.tile([128, HL], F32, tag="ones8")
            nc.vector.memset(ones8, 1.0)

            # ---- phase 1: qkT = [Wq;Wk] @ x^T and V = x @ Wv^T -------
            # (one pass over x^T chunks; V reuses the same xt tiles)
            with (
                tc.tile_pool(name="w1", bufs=1) as w1,
                tc.tile_pool(name="xs1", bufs=1) as xs1,
                tc.tile_pool(name="ps1", bufs=4, space="PSUM") as ps1,
            ):
                wqk = w1.tile([128, KT, 2 * CL], F32R, tag="wqk")
                wv = w1.tile([128, KT, CL], F32R, tag="wv")

                def dma_xt(ci):
                    tiles = [None] * KT
                    for ki in range(KT):
                        tiles[ki] = xs1.tile(
                            [128, 512], F32R, tag=f"xt{ki}", bufs=2,
                            name=f"xt{ki}_{ci}",
                        )
                        nc.sync.dma_start(
                            out=tiles[ki],
                            in_=xT_d[
                                128 * ki : 128 * (ki + 1), 512 * ci : 512 * (ci + 1)
                            ],
                        )
                    return tiles

                # first x chunk before the 12MB of weights; weights m-major so
                # the m=0 accumulation group's strips arrive first
                xt = dma_xt(0)
                for m in range(NM):
                    nc.sync.dma_start(
                        out=wqk[:, :, 128 * m : 128 * (m + 1)],
                        in_=wqk_d[C * m : C * (m + 1), :].rearrange(
                            "(k p) n -> p k n", p=128
                        ),
                    )
                for ki in range(KT):
                    nc.sync.dma_start(
                        out=wv[:, ki, :], in_=wv_d[128 * ki : 128 * (ki + 1), :]
                    )
                for ci in range(NCH):
                    if ci > 0:
                        xt = dma_xt(ci)
                    for m in range(NM):
                        qk_ps = ps1.tile(
                            [128, 512], F32, tag="qk_ps", name=f"qkps{ci}_{m}"
                        )
                        for ki in range(KT):
                            nc.tensor.matmul(
                                qk_ps,
                                lhsT=wqk[:, ki, 128 * m : 128 * (m + 1)],
                                rhs=xt[ki],
                                start=(ki == 0),
                                stop=(ki == KT - 1),
                            )
                        nc.scalar.copy(
                            qkT[:, m, 512 * ci : 512 * (ci + 1)], qk_ps
                        )
                    for tl in range(4):
                        t = 4 * ci + tl
                        v_ps = ps1.tile([128, CL], F32, tag="v_ps", name=f"vps{t}")
                        for ki in range(KT):
                            nc.tensor.matmul(
                                v_ps,
                                lhsT=xt[ki][:, 128 * tl : 128 * (tl + 1)],
                                rhs=wv[:, ki, :],
                                start=(ki == 0),
                                stop=(ki == KT - 1),
                            )
                        vslot = vbuf[:, t, :].rearrange("p (h e) -> p h e", e=HD + 1)
                        nc.scalar.copy(
                            vslot[:, :, 0:HD],
                            v_ps.rearrange("p (h e) -> p h e", e=HD),
                        )
                        nc.vector.tensor_copy(vslot[:, :, HD], ones8)

            # ---- phase 2+3: attention, proj interleaved per chunk ----
            with tc.tile_pool(name="w3", bufs=1) as w3:
                wp = w3.tile([128, NP, C], F32R, tag="wp")
                for p in range(NP):
                    nc.sync.dma_start(
                        out=wp[:, p, :], in_=wp_d[128 * p : 128 * (p + 1), :]
                    )
                ybuf = w3.tile([128, NP, T], F32R, tag="ybuf")  # normalized yT

                with (
                    tc.tile_pool(name="att", bufs=1) as att,
                    tc.tile_pool(name="stg", bufs=3) as stg,
                    tc.tile_pool(name="ps3", bufs=2, space="PSUM") as ps3,
                ):
                    for ci in reversed(range(NCH)):
                        for p in range(NP):
                            pv_ps = ps3.tile(
                                [128, 1024], F32, tag="pv_ps", name=f"pv{p}_{ci}"
                            )
                            nk = 4 * ci + 4  # k-tiles attending into this chunk
                            for ki in range(nk):
                                j = ki - 4 * ci  # >=0 only for diagonal tiles
                                off = 128 * j if j > 0 else 0
                                offs = min(off, 256)  # keep matmul N >= 256
                                s_ps = ps3.tile(
                                    [128, 1024], F32, tag="s_ps",
                                    name=f"s{p}_{ci}_{ki}",
                                )
                                for h2 in range(2):  # even/odd head of the pair
                                    pb = 64 * h2
                                    nc.tensor.matmul(
                                        s_ps[:, 512 * h2 + offs : 512 * (h2 + 1)],
                                        lhsT=qkT[
                                            pb : pb + 64,
                                            4 + p,
                                            128 * ki : 128 * (ki + 1),
                                        ],
                                        rhs=qkT[
                                            pb : pb + 64,
                                            p,
                                            512 * ci + offs : 512 * (ci + 1),
                                        ],
                                        start=True,
                                        stop=True,
                                    )
                                e_t = att.tile(
                                    [128, 2, 512],
                                    F32R,
                                    tag="e_t",
                                    bufs=4,
                                    name=f"e{p}_{ci}_{ki}",
                                )
                                nc.scalar.activation(
                                    e_t[:, :, off:512],
                                    s_ps.rearrange("p (h n) -> p h n", h=2)[
                                        :, :, off:512
                                    ],
                                    EXP,
                                    scale=0.125,
                                )
                                if j >= 0:  # diagonal tile: causal triangle mask
                                    for h2 in range(2):
                                        nc.gpsimd.tensor_mul(
                                            e_t[:, h2, off : off + 128],
                                            e_t[:, h2, off : off + 128],
                                            tri,
                                        )
                                for h2 in range(2):
                                    h = 2 * p + h2
                                    nc.tensor.matmul(
                                        pv_ps[
                                            0 : HD + 1,
                                            512 * h2 + off : 512 * (h2 + 1),
                                        ],
                                        lhsT=vbuf[:, ki, 65 * h : 65 * h + 65],
                                        rhs=e_t[:, h2, off:512],
                                        start=(ki == 0),
                                        stop=(ki == nk - 1),
                                    )
                            # normalize: yT = Y'[0:64] / Y'[64]
                            den = att.tile(
                                [1, 1024], F32, tag="den", bufs=2,
                                name=f"dn{p}_{ci}",
                            )
                            nc.vector.tensor_copy(den, pv_ps[HD : HD + 1, :])
                            recip = att.tile(
                                [1, 1024], F32, tag="recip", bufs=2,
                                name=f"rc{p}_{ci}",
                            )
                            nc.vector.reciprocal_approx_fast(recip, den)
                            bcast = att.tile(
                                [64, 1024], F32, tag="bcast", bufs=2,
                                name=f"bc{p}_{ci}",
                            )
                            nc.gpsimd.partition_broadcast(bcast, recip)
                            for h2 in range(2):
                                nc.vector.tensor_mul(
                                    ybuf[
                                        64 * h2 : 64 * (h2 + 1),
                                        p,
                                        512 * ci : 512 * (ci + 1),
                                    ],
                                    pv_ps[0:HD, 512 * h2 : 512 * h2 + 512],
                                    bcast[:, 512 * h2 : 512 * h2 + 512],
                                )
                        # proj for this chunk's q-tiles (ybuf rows now final)
                        for tl in range(4):
                            t = 4 * ci + tl
                            o_ps = ps3.tile(
                                [128, 1024], F32, tag="pv_ps", name=f"o{t}"
                            )
                            for n in range(2):
                                for p in range(NP):
                                    nc.tensor.matmul(
                                        o_ps[:, 512 * n : 512 * (n + 1)],
                                        lhsT=ybuf[:, p, 128 * t : 128 * (t + 1)],
                                        rhs=wp[:, p, 512 * n : 512 * (n + 1)],
                                        start=(p == 0),
                                        stop=(p == NP - 1),
                                    )
                            o_sb = stg.tile([128, 1024], F32, tag="o_sb", name=f"os{t}")
                            nc.vector.tensor_copy(o_sb, o_ps)
                            nc.sync.dma_start(
                                out=out_d[128 * t : 128 * (t + 1), :], in_=o_sb
                            )

    nc.compile()
    return nc


def _get_module(T=2048):
    if T not in _module_cache:
        _module_cache[T] = build_module(T)
    return _module_cache[T]


def _core_inputs(x, w_attn, w_proj, core):
    b, hg = core // 2, core % 2
    sl = slice(512 * hg, 512 * hg + 512)
    wq = w_attn[0 * C :][sl]
    wk = w_attn[1 * C :][sl]
    wv = w_attn[2 * C :][sl]
    return {
        "xT": np.ascontiguousarray(x[b].T),
        "wqkT": np.ascontiguousarray(
            np.concatenate([wq, wk], axis=0).T.reshape(1024, 8, 128)
            .transpose(1, 0, 2).reshape(8 * 1024, 128)
        ),
        "wvT": np.ascontiguousarray(wv.T),
        "wpT": np.ascontiguousarray(w_proj[:, sl].T),
    }


def kernel(x, w_attn, w_proj):
    global last_results
    x = np.asarray(x, dtype=np.float32)
    w_attn = np.asarray(w_attn, dtype=np.float32)
    w_proj = np.asarray(w_proj, dtype=np.float32)
    B, T, _ = x.shape

    nc = _get_module(T)
    in_maps = [_core_inputs(x, w_attn, w_proj, c) for c in range(N_CORES)]

    kwargs = {}
    if TRACE:
        _install_trace_hook()
        kwargs["trace"] = True
    res = run_bass_kernel_spmd(nc, in_maps, list(range(N_CORES)), **kwargs)
    last_results = res

    out = np.empty((B, T, C), dtype=np.float32)
    for b in range(B):
        out[b] = res.results[2 * b]["out_part"] + res.results[2 * b + 1]["out_part"]
    return out


def _install_trace_hook():
    """The image's antenv lacks axon_hooks; recreate it from trn_boot."""
    import sys
    import types

    if "antenv.axon_hooks" in sys.modules:
        return
    from trn_agent_boot.trn_boot import _ntff_profile_via_ctypes

    hook = _ntff_profile_via_ctypes("/opt/axon/libaxon_pjrt.so")
    mod = types.ModuleType("antenv.axon_hooks")
    mod.get_axon_ntff_profile_hook = lambda: hook
    mod.set_axon_ntff_profile_hook = lambda h: None
    sys.modules["antenv.axon_hooks"] = mod
    import concourse.bass_utils as bu

    bu.upload_artifacts = lambda tmpdir: f"local://{tmpdir}"


# revision 21
# speedup vs baseline: 1.7338x; 1.7338x over previous
"""Causal self-attention (B=4, T=2048, C=1024, H=16) on 8 trn2 NeuronCores.

Sharding: (batch, head-group) -> core.  Core c handles batch b = c//2 and
head-group hg = c%2 (8 heads = 512 channels).  c_attn is column-parallel
(each core computes q/k/v only for its heads), c_proj is row-parallel
(each core computes a partial [T, C] output over its 512 channels); the
host sums the two partials per batch (the unshard step).

Per-core dataflow (all matmul inputs fp32r = full-rate 12-bit-mantissa):
  phase 1a  qkT[ch, T]  = [Wq; Wk] @ x^T          (stationary W, moving x^T)
  phase 1b  V[t, ch]    = x @ Wv^T                (stationary x^T, moving Wv^T)
            V' = [V | 1] per head (ones column -> softmax denominator)
  phase 2   per head-pair, per 512-wide q-chunk:
              S^T[k, q] = K Q^T   (K=64 contraction; even/odd heads at
                                   partition offsets 0/64 -> packed PE tiles)
              E = exp(S^T/8) with causal mask (upper-tri 0/1 multiply)
              Y'^T[65, q] += V'^T E  (row 64 = denominator)
              yT = Y'^T[0:64] * (1/denom)  (reciprocal + partition broadcast)
  phase 3   out[q, C] partial = yT^T @ Wp_slice^T, staged to DRAM.

No collectives; the host-side pairwise add is the row-parallel reduce.
"""

import numpy as np

import concourse.bass as bass
import concourse.mybir as mybir
import concourse.tile as tile
from concourse import bacc
from concourse.bass_utils import run_bass_kernel_spmd

F32 = mybir.dt.float32
F32R = mybir.dt.float32r
EXP = mybir.ActivationFunctionType.Exp

C = 1024  # embed dim
HL = 8  # local heads per core
HD = 64  # head dim
N_CORES = 8

TRACE = False  # set by test harness; enables NTFF profiling
last_results = None  # BassKernelResults of the most recent run

_module_cache = {}


def build_module(T=2048):
    """Emit the per-core Bass program (SPMD; all cores run this)."""
    KT = C // 128  # contraction k-tiles for the projections (8)
    NM = 2 * HL * HD // 128  # qk channel m-tiles (8): 0-3 q pairs, 4-7 k pairs
    TT = T // 128  # T k-tiles (16)
    NCH = T // 512  # 512-wide q chunks (4)
    NP = HL // 2  # head pairs (4)
    CL = HL * HD  # local channels (512)

    nc = bacc.Bacc("TRN2", target_bir_lowering=False, debug=False)

    xT_d = nc.dram_tensor("xT", [C, T], F32R, kind="ExternalInput")
    wqk_d = nc.dram_tensor("wqkT", [NM * C, 128], F32R, kind="ExternalInput")
    wv_d = nc.dram_tensor("wvT", [C, CL], F32R, kind="ExternalInput")
    wp_d = nc.dram_tensor("wpT", [CL, C], F32R, kind="ExternalInput")
    out_d = nc.dram_tensor("out_part", [T, C], F32, kind="ExternalOutput")

    with tile.TileContext(nc) as tc:
        # ---- persistent buffers --------------------------------------
        with tc.tile_pool(name="persist", bufs=1) as cp:
            qkT = cp.tile([128, NM, T], F32R, tag="qkT")  # [ch-tile, m, T]
            vbuf = cp.tile([128, TT, HL * (HD + 1)], F32R, tag="vbuf")  # V'
            tri = cp.tile([128, 128], F32, tag="tri")
            nc.gpsimd.memset(tri, 1.0)
            # keep where (col - p) >= 0: upper triangular incl. diagonal
            nc.gpsimd.affine_select(
                out=tri,
                in_=tri,
                compare_op=mybir.AluOpType.is_ge,
                fill=0.0,
                base=0,
                pattern=[[1, 128]],
                channel_multiplier=-1,
            )
            ones8 = cp.tile([128, HL], F32, tag="ones8")
            nc.vector.memset(ones8, 1.0)

            # ---- phase 1: qkT = [Wq;Wk] @ x^T and V = x @ Wv^T -------
            # (one pass over x^T chunks; V reuses the same xt tiles)
            with (
                tc.tile_pool(name="w1", bufs=1) as w1,
                tc.tile_pool(name="xs1", bufs=1) as xs1,
                tc.tile_pool(name="ps1", bufs=4, space="PSUM") as ps1,
            ):
                wqk = w1.tile([128, KT, 2 * CL], F32R, tag="wqk")
                wv = w1.tile([128, KT, CL], F32R, tag="wv")

                def dma_xt(ci):
                    tiles = [None] * KT
                    for ki in range(KT):
                        tiles[ki] = xs1.tile(
                            [128, 512], F32R, tag=f"xt{ki}", bufs=2,
                            name=f"xt{ki}_{ci}",
                        )
                        nc.sync.dma_start(
                            out=tiles[ki],
                            in_=xT_d[
                                128 * ki : 128 * (ki + 1), 512 * ci : 512 * (ci + 1)
                            ],
                        )
                    return tiles

                # first x chunk before the 12MB of weights; weights m-major so
                # the m=0 accumulation group's strips arrive first
                xt = dma_xt(0)
                for m in range(NM):
                    nc.sync.dma_start(
                        out=wqk[:, :, 128 * m : 128 * (m + 1)],
                        in_=wqk_d[C * m : C * (m + 1), :].rearrange(
                            "(k p) n -> p k n", p=128
                        ),
                    )
                for ki in range(KT):
                    nc.sync.dma_start(
                        out=wv[:, ki, :], in_=wv_d[128 * ki : 128 * (ki + 1), :]
                    )
                for ci in range(NCH):
                    if ci > 0:
                        xt = dma_xt(ci)
                    for m in range(NM):
                        qk_ps = ps1.tile(
                            [128, 512], F32, tag="qk_ps", name=f"qkps{ci}_{m}"
                        )
                        for ki in range(KT):
                            nc.tensor.matmul(
                                qk_ps,
                                lhsT=wqk[:, ki, 128 * m : 128 * (m + 1)],
                                rhs=xt[ki],
                                start=(ki == 0),
                                stop=(ki == KT - 1),
                            )
                        nc.scalar.copy(
                            qkT[:, m, 512 * ci : 512 * (ci + 1)], qk_ps
                        )
                    for tl in range(4):
                        t = 4 * ci + tl
                        v_ps = ps1.tile([128, CL], F32, tag="v_ps", name=f"vps{t}")
                        for ki in range(KT):
                            nc.tensor.matmul(
                                v_ps,
                                lhsT=xt[ki][:, 128 * tl : 128 * (tl + 1)],
                                rhs=wv[:, ki, :],
                                start=(ki == 0),
                                stop=(ki == KT - 1),
                            )
                        vslot = vbuf[:, t, :].rearrange("p (h e) -> p h e", e=HD + 1)
                        nc.scalar.copy(
                            vslot[:, :, 0:HD],
                            v_ps.rearrange("p (h e) -> p h e", e=HD),
                        )
                        nc.vector.tensor_copy(vslot[:, :, HD], ones8)

            # ---- phase 2+3: attention, proj interleaved per chunk ----
            with tc.tile_pool(name="w3", bufs=1) as w3:
                wp = w3.tile([128, NP, C], F32R, tag="wp")
                for p in range(NP):
                    nc.sync.dma_start(
                        out=wp[:, p, :], in_=wp_d[128 * p : 128 * (p + 1), :]
                    )
                ybuf = w3.tile([128, NP, T], F32R, tag="ybuf")  # normalized yT

                with (
                    tc.tile_pool(name="att", bufs=1) as att,
                    tc.tile_pool(name="stg", bufs=3) as stg,
                    tc.tile_pool(name="ps3", bufs=2, space="PSUM") as ps3,
                ):
                    for ci in reversed(range(NCH)):
                        for p in range(NP):
                            pv_ps = ps3.tile(
                                [128, 1024], F32, tag="pv_ps", name=f"pv{p}_{ci}"
                            )
                            nk = 4 * ci + 4  # k-tiles attending into this chunk
                            for ki in range(nk):
                                j = ki - 4 * ci  # >=0 only for diagonal tiles
                                off = 128 * j if j > 0 else 0
                                offs = min(off, 256)  # keep matmul N >= 256
                                s_ps = ps3.tile(
                                    [128, 1024], F32, tag="s_ps",
                                    name=f"s{p}_{ci}_{ki}",
                                )
                                for h2 in range(2):  # even/odd head of the pair
                                    pb = 64 * h2
                                    nc.tensor.matmul(
                                        s_ps[:, 512 * h2 + offs : 512 * (h2 + 1)],
                                        lhsT=qkT[
                                            pb : pb + 64,
                                            4 + p,
                                            128 * ki : 128 * (ki + 1),
                                        ],
                                        rhs=qkT[
                                            pb : pb + 64,
                                            p,
                                            512 * ci + offs : 512 * (ci + 1),
                                        ],
                                        start=True,
                                        stop=True,
                                    )
                                e_t = att.tile(
                                    [128, 2, 512],
                                    F32R,
                                    tag="e_t",
                                    bufs=4,
                                    name=f"e{p}_{ci}_{ki}",
                                )
                                nc.scalar.activation(
                                    e_t[:, :, off:512],
                                    s_ps.rearrange("p (h n) -> p h n", h=2)[
                                        :, :, off:512
                                    ],
                                    EXP,
                                    scale=0.125,
                                )
                                if j >= 0:  # diagonal tile: causal triangle mask
                                    for h2 in range(2):
                                        nc.vector.tensor_mul(
                                            e_t[:, h2, off : off + 128],
                                            e_t[:, h2, off : off + 128],
                                            tri,
                                        )
                                for h2 in range(2):
                                    h = 2 * p + h2
                                    nc.tensor.matmul(
                                        pv_ps[
                                            0 : HD + 1,
                                            512 * h2 + off : 512 * (h2 + 1),
                                        ],
                                        lhsT=vbuf[:, ki, 65 * h : 65 * h + 65],
                                        rhs=e_t[:, h2, off:512],
                                        start=(ki == 0),
                                        stop=(ki == nk - 1),
                                    )
                            # normalize: yT = Y'[0:64] / Y'[64]
                            den = att.tile(
                                [1, 1024], F32, tag="den", bufs=2,
                                name=f"dn{p}_{ci}",
                            )
                            nc.vector.tensor_copy(den, pv_ps[HD : HD + 1, :])
                            recip = att.tile(
                                [1, 1024], F32, tag="recip", bufs=2,
                                name=f"rc{p}_{ci}",
                            )
                            nc.vector.reciprocal_approx_fast(recip, den)
                            bcast = att.tile(
                                [64, 1024], F32, tag="bcast", bufs=2,
                                name=f"bc{p}_{ci}",
                            )
                            nc.gpsimd.partition_broadcast(bcast, recip)
                            for h2 in range(2):
                                nc.vector.tensor_mul(
                                    ybuf[
                                        64 * h2 : 64 * (h2 + 1),
                                        p,
                                        512 * ci : 512 * (ci + 1),
                                    ],
                                    pv_ps[0:HD, 512 * h2 : 512 * h2 + 512],
                                    bcast[:, 512 * h2 : 512 * h2 + 512],
                                )
                        # proj for this chunk's q-tiles (ybuf rows now final)
                        for tl in range(4):
                            t = 4 * ci + tl
                            o_ps = ps3.tile(
                                [128, 1024], F32, tag="pv_ps", name=f"o{t}"
                            )
                            for n in range(2):
                                for p in range(NP):
                                    nc.tensor.matmul(
                                        o_ps[:, 512 * n : 512 * (n + 1)],
                                        lhsT=ybuf[:, p, 128 * t : 128 * (t + 1)],
                                        rhs=wp[:, p, 512 * n : 512 * (n + 1)],
                                        start=(p == 0),
                                        stop=(p == NP - 1),
                                    )
                            o_sb = stg.tile([128, 1024], F32, tag="o_sb", name=f"os{t}")
                            nc.vector.tensor_copy(o_sb, o_ps)
                            nc.sync.dma_start(
                                out=out_d[128 * t : 128 * (t + 1), :], in_=o_sb
                            )

    nc.compile()
    return nc


def _get_module(T=2048):
    if T not in _module_cache:
        _module_cache[T] = build_module(T)
    return _module_cache[T]


def _core_inputs(x, w_attn, w_proj, core):
    b, hg = core // 2, core % 2
    sl = slice(512 * hg, 512 * hg + 512)
    wq = w_attn[0 * C :][sl]
    wk = w_attn[1 * C :][sl]
    wv = w_attn[2 * C :][sl]
    return {
        "xT": np.ascontiguousarray(x[b].T),
        "wqkT": np.ascontiguousarray(
            np.concatenate([wq, wk], axis=0).T.reshape(1024, 8, 128)
            .transpose(1, 0, 2).reshape(8 * 1024, 128)
        ),
        "wvT": np.ascontiguousarray(wv.T),
        "wpT": np.ascontiguousarray(w_proj[:, sl].T),
    }


def kernel(x, w_attn, w_proj):
    global last_results
    x = np.asarray(x, dtype=np.float32)
    w_attn = np.asarray(w_attn, dtype=np.float32)
    w_proj = np.asarray(w_proj, dtype=np.float32)
    B, T, _ = x.shape

    nc = _get_module(T)
    in_maps = [_core_inputs(x, w_attn, w_proj, c) for c in range(N_CORES)]

    kwargs = {}
    if TRACE:
        _install_trace_hook()
        kwargs["trace"] = True
    res = run_bass_kernel_spmd(nc, in_maps, list(range(N_CORES)), **kwargs)
    last_results = res

    out = np.empty((B, T, C), dtype=np.float32)
    for b in range(B):
        out[b] = res.results[2 * b]["out_part"] + res.results[2 * b + 1]["out_part"]
    return out


def _install_trace_hook():
    """The image's antenv lacks axon_hooks; recreate it from trn_boot."""
    import sys
    import types

    if "antenv.axon_hooks" in sys.modules:
        return
    from trn_agent_boot.trn_boot import _ntff_profile_via_ctypes

    hook = _ntff_profile_via_ctypes("/opt/axon/libaxon_pjrt.so")
    mod = types.ModuleType("antenv.axon_hooks")
    mod.get_axon_ntff_profile_hook = lambda: hook
    mod.set_axon_ntff_profile_hook = lambda h: None
    sys.modules["antenv.axon_hooks"] = mod
    import concourse.bass_utils as bu

    bu.upload_artifacts = lambda tmpdir: f"local://{tmpdir}"


# revision 23
# speedup vs baseline: 1.7345x; 1.0004x over previous
"""Causal self-attention (B=4, T=2048, C=1024, H=16) on 8 trn2 NeuronCores.

Sharding: (batch, head-group) -> core.  Core c handles batch b = c//2 and
head-group hg = c%2 (8 heads = 512 channels).  c_attn is column-parallel
(each core computes q/k/v only for its heads), c_proj is row-parallel
(each core computes a partial [T, C] output over its 512 channels); the
host sums the two partials per batch (the unshard step).

Per-core dataflow (all matmul inputs fp32r = full-rate 12-bit-mantissa):
  phase 1a  qkT[ch, T]  = [Wq; Wk] @ x^T          (stationary W, moving x^T)
  phase 1b  V[t, ch]    = x @ Wv^T                (stationary x^T, moving Wv^T)
            V' = [V | 1] per head (ones column -> softmax denominator)
  phase 2   per head-pair, per 512-wide q-chunk:
              S^T[k, q] = K Q^T   (K=64 contraction; even/odd heads at
                                   partition offsets 0/64 -> packed PE tiles)
              E = exp(S^T/8) with causal mask (upper-tri 0/1 multiply)
              Y'^T[65, q] += V'^T E  (row 64 = denominator)
              yT = Y'^T[0:64] * (1/denom)  (reciprocal + partition broadcast)
  phase 3   out[q, C] partial = yT^T @ Wp_slice^T, staged to DRAM.

No collectives; the host-side pairwise add is the row-parallel reduce.
"""

import numpy as np

import concourse.bass as bass
import concourse.mybir as mybir
import concourse.tile as tile
from concourse import bacc
from concourse.bass_utils import run_bass_kernel_spmd

F32 = mybir.dt.float32
F32R = mybir.dt.float32r
EXP = mybir.ActivationFunctionType.Exp

C = 1024  # embed dim
HL = 8  # local heads per core
HD = 64  # head dim
N_CORES = 8

TRACE = False  # set by test harness; enables NTFF profiling
last_results = None  # BassKernelResults of the most recent run

_module_cache = {}


def build_module(T=2048):
    """Emit the per-core Bass program (SPMD; all cores run this)."""
    KT = C // 128  # contraction k-tiles for the projections (8)
    NM = 2 * HL * HD // 128  # qk channel m-tiles (8): 0-3 q pairs, 4-7 k pairs
    TT = T // 128  # T k-tiles (16)
    NCH = T // 512  # 512-wide q chunks (4)
    NP = HL // 2  # head pairs (4)
    CL = HL * HD  # local channels (512)

    nc = bacc.Bacc("TRN2", target_bir_lowering=False, debug=False)

    xT_d = nc.dram_tensor("xT", [C, T], F32R, kind="ExternalInput")
    wqk_d = nc.dram_tensor("wqkT", [NM * C, 128], F32R, kind="ExternalInput")
    wv_d = nc.dram_tensor("wvT", [C, CL], F32R, kind="ExternalInput")
    wp_d = nc.dram_tensor("wpT", [CL, C], F32R, kind="ExternalInput")
    out_d = nc.dram_tensor("out_part", [T, C], F32, kind="ExternalOutput")

    with tile.TileContext(nc) as tc:
        # ---- persistent buffers --------------------------------------
        with tc.tile_pool(name="persist", bufs=1) as cp:
            qkT = cp.tile([128, NM, T], F32R, tag="qkT")  # [ch-tile, m, T]
            vbuf = cp.tile([128, TT, HL * (HD + 1)], F32R, tag="vbuf")  # V'
            tri = cp.tile([128, 128], F32, tag="tri")
            nc.gpsimd.memset(tri, 1.0)
            # keep where (col - p) >= 0: upper triangular incl. diagonal
            nc.gpsimd.affine_select(
                out=tri,
                in_=tri,
                compare_op=mybir.AluOpType.is_ge,
                fill=0.0,
                base=0,
                pattern=[[1, 128]],
                channel_multiplier=-1,
            )
            ones8 = cp.tile([128, HL], F32, tag="ones8")
            nc.vector.memset(ones8, 1.0)

            # ---- phase 1: qkT = [Wq;Wk] @ x^T and V = x @ Wv^T -------
            # (one pass over x^T chunks; V reuses the same xt tiles)
            with (
                tc.tile_pool(name="w1", bufs=1) as w1,
                tc.tile_pool(name="xs1", bufs=1) as xs1,
                tc.tile_pool(name="ps1", bufs=4, space="PSUM") as ps1,
            ):
                wqk = w1.tile([128, KT, 2 * CL], F32R, tag="wqk")
                wv = w1.tile([128, KT, CL], F32R, tag="wv")

                def dma_xt(ci):
                    tiles = [None] * KT
                    for ki in range(KT):
                        tiles[ki] = xs1.tile(
                            [128, 512], F32R, tag=f"xt{ki}", bufs=2,
                            name=f"xt{ki}_{ci}",
                        )
                        nc.sync.dma_start(
                            out=tiles[ki],
                            in_=xT_d[
                                128 * ki : 128 * (ki + 1), 512 * ci : 512 * (ci + 1)
                            ],
                        )
                    return tiles

                # first x chunk before the 12MB of weights; weights m-major so
                # the m=0 accumulation group's strips arrive first
                xt = dma_xt(0)
                for m in range(NM):
                    nc.sync.dma_start(
                        out=wqk[:, :, 128 * m : 128 * (m + 1)],
                        in_=wqk_d[C * m : C * (m + 1), :].rearrange(
                            "(k p) n -> p k n", p=128
                        ),
                    )
                for ki in range(KT):
                    nc.sync.dma_start(
                        out=wv[:, ki, :], in_=wv_d[128 * ki : 128 * (ki + 1), :]
                    )
                for ci in range(NCH):
                    if ci > 0:
                        xt = dma_xt(ci)
                    for m in range(NM):
                        qk_ps = ps1.tile(
                            [128, 512], F32, tag="qk_ps", name=f"qkps{ci}_{m}"
                        )
                        for ki in range(KT):
                            nc.tensor.matmul(
                                qk_ps,
                                lhsT=wqk[:, ki, 128 * m : 128 * (m + 1)],
                                rhs=xt[ki],
                                start=(ki == 0),
                                stop=(ki == KT - 1),
                            )
                        nc.scalar.copy(
                            qkT[:, m, 512 * ci : 512 * (ci + 1)], qk_ps
                        )
                    for tl in range(4):
                        t = 4 * ci + tl
                        v_ps = ps1.tile([128, CL], F32, tag="v_ps", name=f"vps{t}")
                        for ki in range(KT):
                            nc.tensor.matmul(
                                v_ps,
                                lhsT=xt[ki][:, 128 * tl : 128 * (tl + 1)],
                                rhs=wv[:, ki, :],
                                start=(ki == 0),
                                stop=(ki == KT - 1),
                            )
                        vslot = vbuf[:, t, :].rearrange("p (h e) -> p h e", e=HD + 1)
                        nc.scalar.copy(
                            vslot[:, :, 0:HD],
                            v_ps.rearrange("p (h e) -> p h e", e=HD),
                        )
                        nc.vector.tensor_copy(vslot[:, :, HD], ones8)

            # ---- phase 2+3: attention, proj interleaved per chunk ----
            with tc.tile_pool(name="w3", bufs=1) as w3:
                wp = w3.tile([128, NP, C], F32R, tag="wp")
                for p in range(NP):
                    nc.sync.dma_start(
                        out=wp[:, p, :], in_=wp_d[128 * p : 128 * (p + 1), :]
                    )
                ybuf = w3.tile([128, NP, T], F32R, tag="ybuf")  # normalized yT

                with (
                    tc.tile_pool(name="att", bufs=1) as att,
                    tc.tile_pool(name="stg", bufs=3) as stg,
                    tc.tile_pool(name="ps3", bufs=2, space="PSUM") as ps3,
                ):
                    for ci in reversed(range(NCH)):
                        for p in range(NP):
                            pv_ps = ps3.tile(
                                [128, 1024], F32, tag="pv_ps", name=f"pv{p}_{ci}"
                            )
                            nk = 4 * ci + 4  # k-tiles attending into this chunk
                            for ki in range(nk):
                                j = ki - 4 * ci  # >=0 only for diagonal tiles
                                off = 128 * j if j > 0 else 0
                                offs = min(off, 256)  # keep matmul N >= 256
                                s_ps = ps3.tile(
                                    [128, 1024], F32, tag="s_ps",
                                    name=f"s{p}_{ci}_{ki}",
                                )
                                for h2 in range(2):  # even/odd head of the pair
                                    pb = 64 * h2
                                    nc.tensor.matmul(
                                        s_ps[:, 512 * h2 + offs : 512 * (h2 + 1)],
                                        lhsT=qkT[
                                            pb : pb + 64,
                                            4 + p,
                                            128 * ki : 128 * (ki + 1),
                                        ],
                                        rhs=qkT[
                                            pb : pb + 64,
                                            p,
                                            512 * ci + offs : 512 * (ci + 1),
                                        ],
                                        start=True,
                                        stop=True,
                                    )
                                e_t = att.tile(
                                    [128, 2, 512],
                                    F32R,
                                    tag="e_t",
                                    bufs=4,
                                    name=f"e{p}_{ci}_{ki}",
                                )
                                nc.scalar.activation(
                                    e_t[:, :, off:512],
                                    s_ps.rearrange("p (h n) -> p h n", h=2)[
                                        :, :, off:512
                                    ],
                                    EXP,
                                    scale=0.125,
                                )
                                if j >= 0:  # diagonal tile: causal triangle mask
                                    for h2 in range(2):
                                        nc.vector.tensor_mul(
                                            e_t[:, h2, off : off + 128],
                                            e_t[:, h2, off : off + 128],
                                            tri,
                                        )
                                for h2 in range(2):
                                    h = 2 * p + h2
                                    nc.tensor.matmul(
                                        pv_ps[
                                            0 : HD + 1,
                                            512 * h2 + off : 512 * (h2 + 1),
                                        ],
                                        lhsT=vbuf[:, ki, 65 * h : 65 * h + 65],
                                        rhs=e_t[:, h2, off:512],
                                        start=(ki == 0),
                                        stop=(ki == nk - 1),
                                    )
                            # normalize: yT = Y'[0:64] / Y'[64]
                            den = att.tile(
                                [1, 1024], F32, tag="den", bufs=2,
                                name=f"dn{p}_{ci}",
                            )
                            nc.vector.tensor_copy(den, pv_ps[HD : HD + 1, :])
                            recip = att.tile(
                                [1, 1024], F32, tag="recip", bufs=2,
                                name=f"rc{p}_{ci}",
                            )
                            nc.vector.reciprocal_approx_fast(recip, den)
                            bcast = att.tile(
                                [64, 1024], F32, tag="bcast", bufs=2,
                                name=f"bc{p}_{ci}",
                            )
                            nc.gpsimd.partition_broadcast(bcast, recip)
                            for h2 in range(2):
                                nc.vector.tensor_mul(
                                    ybuf[
                                        64 * h2 : 64 * (h2 + 1),
                                        p,
                                        512 * ci : 512 * (ci + 1),
                                    ],
                                    pv_ps[0:HD, 512 * h2 : 512 * h2 + 512],
                                    bcast[:, 512 * h2 : 512 * h2 + 512],
                                )
                        # proj for this chunk's q-tiles (ybuf rows now final)
                        for tl in range(4):
                            t = 4 * ci + tl
                            o_ps = ps3.tile(
                                [128, 1024], F32, tag="pv_ps", name=f"o{t}"
                            )
                            for n in range(2):
                                for p in range(NP):
                                    nc.tensor.matmul(
                                        o_ps[:, 512 * n : 512 * (n + 1)],
                                        lhsT=ybuf[:, p, 128 * t : 128 * (t + 1)],
                                        rhs=wp[:, p, 512 * n : 512 * (n + 1)],
                                        start=(p == 0),
                                        stop=(p == NP - 1),
                                    )
                            o_sb = stg.tile([128, 1024], F32, tag="o_sb", name=f"os{t}")
                            nc.vector.tensor_copy(o_sb, o_ps)
                            nc.sync.dma_start(
                                out=out_d[128 * t : 128 * (t + 1), :], in_=o_sb
                            )

    nc.compile()
    return nc


def _get_module(T=2048):
    if T not in _module_cache:
        _module_cache[T] = build_module(T)
    return _module_cache[T]


def _core_inputs(x, w_attn, w_proj, core):
    b, hg = core // 2, core % 2
    sl = slice(512 * hg, 512 * hg + 512)
    wq = w_attn[0 * C :][sl]
    wk = w_attn[1 * C :][sl]
    wv = w_attn[2 * C :][sl]
    return {
        "xT": np.ascontiguousarray(x[b].T),
        "wqkT": np.ascontiguousarray(
            np.concatenate([wq, wk], axis=0).T.reshape(1024, 8, 128)
            .transpose(1, 0, 2).reshape(8 * 1024, 128)
        ),
        "wvT": np.ascontiguousarray(wv.T),
        "wpT": np.ascontiguousarray(w_proj[:, sl].T),
    }


def kernel(x, w_attn, w_proj):
    global last_results
    x = np.asarray(x, dtype=np.float32)
    w_attn = np.asarray(w_attn, dtype=np.float32)
    w_proj = np.asarray(w_proj, dtype=np.float32)
    B, T, _ = x.shape

    nc = _get_module(T)
    in_maps = [_core_inputs(x, w_attn, w_proj, c) for c in range(N_CORES)]

    kwargs = {}
    if TRACE:
        _install_trace_hook()
        kwargs["trace"] = True
    res = run_bass_kernel_spmd(nc, in_maps, list(range(N_CORES)), **kwargs)
    last_results = res

    out = np.empty((B, T, C), dtype=np.float32)
    for b in range(B):
        out[b] = res.results[2 * b]["out_part"] + res.results[2 * b + 1]["out_part"]
    return out


def _install_trace_hook():
    """The image's antenv lacks axon_hooks; recreate it from trn_boot."""
    import sys
    import types

    if "antenv.axon_hooks" in sys.modules:
        return
    from trn_agent_boot.trn_boot import _ntff_profile_via_ctypes

    hook = _ntff_profile_via_ctypes("/opt/axon/libaxon_pjrt.so")
    mod = types.ModuleType("antenv.axon_hooks")
    mod.get_axon_ntff_profile_hook = lambda: hook
    mod.set_axon_ntff_profile_hook = lambda h: None
    sys.modules["antenv.axon_hooks"] = mod
    import concourse.bass_utils as bu

    bu.upload_artifacts = lambda tmpdir: f"local://{tmpdir}"
